# revision 14
# baseline (speedup 1.0000x reference)
"""FESTGCN Trainium2 kernel: 8-core SPMD Bass/Tile implementation (v4).

Algorithm (validated against the reference in numpy, see sim_check.py):
  For t in 0..9:
    M_t = dtw * (ceil|td| > 9-t) + (spec_lap + I)   [t=9: + laplacian, x2/3]
    S1 += M_t @ c1_t,   c1_t = [x_t | h_t]
    gcn1_t = 0.5 * S1 @ W1 + (t+1) b1 ;  sig_t = sigmoid(gcn1_t)
    r_t = first flat half of sig_t ;  c2_t = [x_t | r_t*h_t]
    S2 += M_t @ c2_t
  u = second flat half of sig_9 ; c = tanh(0.5 * S2 @ W2 + 10 b2)
  out = u*h_9 + (1-u)*c        (final mix done on host)

Structure:
  * All ten masked matrices M_t are baked on the HOST (fp16, transposed and
    sliced per core, laid out [nt, t, 128, 512] so two consecutive steps load
    as one 256KB DMA) - TRN2's vector engine is far too slow to mask per step.
  * Contraction axis in sigma order (even c2-nodes first); each core owns rows
    pi_c = [256c,+256) u [2048+256c,+256), so the per-step AllGather carries
    only the r-side sigmoids and every gather read is a contiguous row slice.
  * The sigmoid vector is widened 64->66 per batch with two always-1.0
    channels (bias 30 through sigmoid), so c2 = [x | r*h] is ONE elementwise
    multiply of the gathered gate slice with the c1 tile - no extra x column
    handling on any engine.
  * Single interleaved pass: conv2(t) runs DELAY=2 outer steps behind
    conv1(t); M/c1 tiles are held in SBUF across the window, AllGather latency
    hides under the intervening compute.  A dummy AllGather up front absorbs
    the collective entry barrier.
  * Final gating mix on host (no aux collective, no redundant full output).
"""

import numpy as np

import concourse.bacc as bacc
import concourse.mybir as mybir
import concourse.tile as tile
from concourse.bass_utils import run_bass_kernel_spmd

B, T, N, H = 4, 10, 4096, 32
NC = 8
RPC = N // NC            # 512 rows per core
NT = N // 128            # 32 contraction tiles
MT = RPC // 128          # 4 m-tiles per core
F1 = B * (H + 1)         # 132 moving columns per conv
K2 = 2 * H + 2           # 66: widened sigmoid channels per batch
FS = B * K2              # 264
NH = N * H
DELAY = 2                # conv2 runs this many outer steps behind conv1
f32 = mybir.dt.float32
f16 = mybir.dt.float16
Alu = mybir.AluOpType
Act = mybir.ActivationFunctionType
CORES = list(range(NC))

PERM = np.concatenate([np.arange(0, N, 2), np.arange(1, N, 2)])
IPERM = np.empty(N, np.int64)
IPERM[PERM] = np.arange(N)


def _rows_of(c):
    return np.concatenate(
        [np.arange(256 * c, 256 * c + 256),
         np.arange(2048 + 256 * c, 2048 + 256 * c + 256)]
    )


def _build_nc():
    nc = bacc.Bacc(
        "TRN2",
        target_bir_lowering=False,
        debug=False,
        enable_asserts=False,
        num_devices=NC,
    )
    mstack = nc.dram_tensor("mstack", [NT, T, 128, RPC], f16,
                            kind="ExternalInput").ap()
    c1all = nc.dram_tensor("c1all", [T, N, F1], f16, kind="ExternalInput").ap()
    w1h = nc.dram_tensor("w1h", [H + 1, K2], f32, kind="ExternalInput").ap()
    w2h = nc.dram_tensor("w2h", [H + 1, H], f32, kind="ExternalInput").ap()
    biastab = nc.dram_tensor("biastab", [11, FS], f32, kind="ExternalInput").ap()
    hout = nc.dram_tensor("hout", [RPC, B * H], f32, kind="ExternalOutput").ap()
    sig9 = nc.dram_tensor("sig9", [256, FS], f16, kind="ExternalOutput").ap()

    with tile.TileContext(nc) as tc:
        with (
            tc.tile_pool(name="msp", bufs=64) as msp,       # M step-pair tiles
            tc.tile_pool(name="c1p", bufs=100) as c1p,      # c1 tiles, 3-step hold
            tc.tile_pool(name="rlp", bufs=36) as rlp,       # gathered sigmoids
            tc.tile_pool(name="c2p", bufs=8) as c2p,
            tc.tile_pool(name="sm", bufs=1) as sm,
            tc.tile_pool(name="acc", bufs=3) as accp,
            tc.tile_pool(name="wk", bufs=3) as wk,
            tc.tile_pool(name="zp", bufs=1, space="PSUM") as zp,
            tc.tile_pool(name="tpz", bufs=2, space="PSUM") as tpzp,
            tc.tile_pool(name="g1p", bufs=2, space="PSUM") as g1p,
            tc.tile_pool(name="dramp", bufs=1, space="DRAM") as dramp,
        ):
            agsrc = [
                dramp.tile([256, FS], f16, tag=f"agsrc{t}", name=f"agsrc{t}")
                for t in range(T)
            ]
            agdst = [
                dramp.tile([NC * 256, FS], f16, tag=f"agdst{t}",
                           name=f"agdst{t}", addr_space="Shared")
                for t in range(T)
            ]
            agwsrc = dramp.tile([8, 8], f16, tag="agwsrc", name="agwsrc")
            agwdst = dramp.tile([NC * 8, 8], f16, tag="agwdst", name="agwdst",
                                addr_space="Shared")

            # ---------------- prologue ----------------
            iota_i = wk.tile([128, 128], mybir.dt.int32, tag="iota", bufs=1)
            nc.gpsimd.iota(iota_i[:], pattern=[[1, 128]], base=0,
                           channel_multiplier=-1)
            ident = sm.tile([128, 128], f32, tag="ident")
            nc.vector.tensor_scalar(ident[:], iota_i[:], 0, None,
                                    op0=Alu.is_equal)
            w1s = sm.tile([H + 1, K2], f32, tag="w1s")
            nc.sync.dma_start(w1s[:], w1h[:])
            w2s = sm.tile([H + 1, H], f32, tag="w2s")
            nc.sync.dma_start(w2s[:], w2h[:])
            wup = wk.tile([8, 8], f16, tag="wup", bufs=1)
            nc.vector.memset(wup[:], 0.0)
            nc.sync.dma_start(agwsrc[:], wup[:])
            # dummy collective: absorbs the CC entry barrier before step 0
            nc.gpsimd.collective_compute(
                "AllGather", Alu.bypass, replica_groups=[CORES],
                ins=[agwsrc[:]], outs=[agwdst[:]],
            )

            s1 = [accp.tile([128, F1], f32, tag=f"s1_{mt}", name=f"s1_{mt}")
                  for mt in range(MT)]
            s2 = [accp.tile([128, F1], f32, tag=f"s2_{mt}", name=f"s2_{mt}")
                  for mt in range(MT)]
            for mt in range(MT):
                nc.vector.memset(s1[mt][:], 0.0)
                nc.vector.memset(s2[mt][:], 0.0)

            ms_hold = {}
            c1_hold = {}
            rl_hold = {}

            def chain1(t, s1t):
                """S1(t) -> gcn1 -> sigmoid (widened) -> agsrc[t]/AG."""
                mts = (0, 1, 2, 3) if t == T - 1 else (0, 1)
                biasf = wk.tile([128, FS], f32, tag="biasf", bufs=3)
                nc.sync.dma_start(
                    biasf[:], biastab[t : t + 1, :].broadcast_to((128, FS))
                )
                for mt in mts:
                    tb = wk.tile([128, F1], f32, tag="tbf", bufs=4)
                    nc.vector.tensor_copy(tb[:], s1t[mt][:])
                    g1 = g1p.tile([128, FS], f32, tag="g1")
                    for b in range(B):
                        tz = tpzp.tile([H + 1, 128], f32, tag="tz")
                        nc.tensor.transpose(
                            tz[:], tb[:, b * (H + 1) : (b + 1) * (H + 1)],
                            ident[:],
                        )
                        zbt = wk.tile([H + 1, 128], f32, tag="zbt", bufs=4)
                        nc.scalar.copy(zbt[:], tz[:])
                        nc.tensor.matmul(
                            g1[:, b * K2 : (b + 1) * K2], zbt[:], w1s[:],
                            start=True, stop=True,
                        )
                    sigi = wk.tile([128, FS], f32, tag="sigi", bufs=4)
                    nc.vector.scalar_tensor_tensor(
                        sigi[:], g1[:], 1.0, biasf[:], op0=Alu.mult, op1=Alu.add
                    )
                    sigb = wk.tile([128, FS], f16, tag="sigb", bufs=4)
                    nc.scalar.activation(sigb[:], sigi[:], Act.Sigmoid)
                    if mt < 2:
                        nc.sync.dma_start(
                            agsrc[t][mt * 128 : (mt + 1) * 128, :], sigb[:]
                        )
                    else:
                        nc.sync.dma_start(
                            sig9[(mt - 2) * 128 : (mt - 1) * 128, :], sigb[:]
                        )
                nc.gpsimd.collective_compute(
                    "AllGather",
                    Alu.bypass,
                    replica_groups=[CORES],
                    ins=[agsrc[t][:]],
                    outs=[agdst[t][:]],
                )

            def fetch_rl(t):
                # One outer step after AG(t) fired, so the scalar queue's
                # semaphore wait on the collective is already satisfied.
                rls = []
                for gt in range(16):
                    r = rlp.tile([128, FS], f16, tag="rl", name="rl")
                    nc.scalar.dma_start(
                        r[:], agdst[t][gt * 128 : (gt + 1) * 128, :]
                    )
                    rls.append(r)
                rl_hold[t] = rls

            # ---------------- interleaved main loop ----------------
            for s in range(T + DELAY):
                if s < T:
                    t = s
                    za = zp.tile([128, 2 * F1], f32, tag="z1a", name=f"z1a_{t}")
                    zb = zp.tile([128, 2 * F1], f32, tag="z1b", name=f"z1b_{t}")
                    zh = [za, zb]
                    if t % 2 == 0:
                        mss = []
                        for nt in range(NT):
                            m = msp.tile([128, 2 * RPC], f16, tag="ms",
                                         name="ms")
                            eng = nc.sync if nt < 24 else nc.scalar
                            eng.dma_start(
                                m.rearrange("p (two m) -> p two m", two=2),
                                mstack[nt, t : t + 2, :, :].rearrange(
                                    "two p m -> p two m"
                                ),
                            )
                            mss.append(m)
                        ms_hold[t] = mss
                        ms_hold[t + 1] = mss
                    mss = ms_hold[t]
                    moff = (t % 2) * RPC
                    c1s = []
                    for nt in range(NT):
                        c1 = c1p.tile([128, F1], f16, tag="c1", name="c1")
                        nc.scalar.dma_start(
                            c1[:], c1all[t, nt * 128 : (nt + 1) * 128, :]
                        )
                        c1s.append(c1)
                        for mt in range(MT):
                            nc.tensor.matmul(
                                zh[mt // 2][:, (mt % 2) * F1 : (mt % 2 + 1) * F1],
                                mss[nt][:, moff + mt * 128 : moff + (mt + 1) * 128],
                                c1[:],
                                start=(nt == 0 and mt % 2 == 0),
                                stop=(nt == NT - 1),
                            )
                    c1_hold[t] = c1s
                    s1t = []
                    for mt in range(MT):
                        s1n = accp.tile([128, F1], f32, tag=f"s1_{mt}")
                        nc.vector.tensor_add(
                            s1n[:], s1[mt][:],
                            zh[mt // 2][:, (mt % 2) * F1 : (mt % 2 + 1) * F1],
                        )
                        s1[mt] = s1n
                        s1t.append(s1n)
                    chain1(t, s1t)
                if 1 <= s <= T:
                    fetch_rl(s - 1)

                if s >= DELAY:
                    t2 = s - DELAY
                    za = zp.tile([128, 2 * F1], f32, tag="z2a", name=f"z2a_{t2}")
                    zb = zp.tile([128, 2 * F1], f32, tag="z2b", name=f"z2b_{t2}")
                    zh = [za, zb]
                    mss = ms_hold.pop(t2)
                    moff = (t2 % 2) * RPC
                    c1s = c1_hold.pop(t2)
                    rls = rl_hold.pop(t2)
                    for nt in range(NT):
                        gt, p = (nt, 0) if nt < 16 else (nt - 16, 1)
                        c2 = c2p.tile([128, F1], f16, tag="c2", name="c2")
                        nc.vector.tensor_mul(
                            c2.rearrange("p (b k) -> p b k", k=H + 1),
                            rls[gt].rearrange("p (b k) -> p b k", k=K2)[
                                :, :, 33 * p : 33 * p + 33
                            ],
                            c1s[nt].rearrange("p (b k) -> p b k", k=H + 1),
                        )
                        for mt in range(MT):
                            nc.tensor.matmul(
                                zh[mt // 2][:, (mt % 2) * F1 : (mt % 2 + 1) * F1],
                                mss[nt][:, moff + mt * 128 : moff + (mt + 1) * 128],
                                c2[:],
                                start=(nt == 0 and mt % 2 == 0),
                                stop=(nt == NT - 1),
                            )
                    for mt in range(MT):
                        s2n = accp.tile([128, F1], f32, tag=f"s2_{mt}")
                        nc.vector.tensor_add(
                            s2n[:], s2[mt][:],
                            zh[mt // 2][:, (mt % 2) * F1 : (mt % 2 + 1) * F1],
                        )
                        s2[mt] = s2n

            # ---------------- tail ----------------
            bias2f = wk.tile([128, B * H], f32, tag="bias2f", bufs=1)
            nc.sync.dma_start(
                bias2f[:], biastab[10 : 11, : B * H].broadcast_to((128, B * H))
            )
            for mt in range(MT):
                tb2 = wk.tile([128, F1], f32, tag="tbf", bufs=4)
                nc.vector.tensor_copy(tb2[:], s2[mt][:])
                g2 = g1p.tile([128, FS], f32, tag="g1")
                for b in range(B):
                    tz = tpzp.tile([H + 1, 128], f32, tag="tz")
                    nc.tensor.transpose(
                        tz[:], tb2[:, b * (H + 1) : (b + 1) * (H + 1)], ident[:]
                    )
                    zbt = wk.tile([H + 1, 128], f32, tag="zbt", bufs=4)
                    nc.scalar.copy(zbt[:], tz[:])
                    nc.tensor.matmul(
                        g2[:, b * H : (b + 1) * H], zbt[:], w2s[:],
                        start=True, stop=True,
                    )
                tani = wk.tile([128, B * H], f32, tag="tani", bufs=2)
                nc.vector.scalar_tensor_tensor(
                    tani[:], g2[:, : B * H], 1.0, bias2f[:],
                    op0=Alu.mult, op1=Alu.add,
                )
                tanf = wk.tile([128, B * H], f32, tag="tanf", bufs=2)
                nc.scalar.activation(tanf[:], tani[:], Act.Tanh)
                nc.sync.dma_start(hout[mt * 128 : (mt + 1) * 128, :], tanf[:])

    nc.finalize()
    return nc


_NC_CACHE = None


def _get_nc():
    global _NC_CACHE
    if _NC_CACHE is None:
        _NC_CACHE = _build_nc()
    return _NC_CACHE


def make_in_maps(inputs, states, dtw, spec_lap, laplacian, time_delay,
                 W1, b1, W2, b2):
    inputs = np.asarray(inputs, np.float32)
    states = np.asarray(states, np.float32)
    dtw = np.asarray(dtw, np.float32)
    spec_lap = np.asarray(spec_lap, np.float32)
    laplacian = np.asarray(laplacian, np.float32)
    time_delay = np.asarray(time_delay, np.float32)
    W1 = np.asarray(W1, np.float32)
    b1 = np.asarray(b1, np.float32)
    W2 = np.asarray(W2, np.float32)
    b2 = np.asarray(b2, np.float32)

    ct_full = np.ceil(np.abs(time_delay))
    Gdtw = np.ascontiguousarray(dtw[:, PERM].T)
    Gct = np.ascontiguousarray(ct_full[:, PERM].T)
    Gsle = spec_lap[:, PERM].T.copy()
    Gsle[IPERM, np.arange(N)] += 1.0
    Glap = laplacian[:, PERM].T

    scratch = np.empty((N, N), np.float32)
    mst = np.empty((T, N, N), np.float16)
    for t in range(T - 1):
        np.multiply(Gdtw, (Gct > np.float32(9 - t)), out=scratch)
        scratch += Gsle
        mst[t] = scratch
    np.multiply(Gdtw, (Gct >= np.float32(1.0)), out=scratch)
    scratch += Gsle
    scratch += Glap
    mst[T - 1] = scratch

    x = inputs.transpose(1, 0, 2)               # [T, B, N]
    h = states.reshape(T, B, N, H)
    conc = np.concatenate([x[:, :, :, None], h], axis=3)  # [T,B,N,33]
    conc[9] *= 2.0 / 3.0
    concp = conc.transpose(0, 2, 1, 3)[:, PERM]  # [T, N(sigma), B, 33]
    c1all = np.ascontiguousarray(concp.reshape(T, N, F1), np.float16)

    # widened W1 / biases: channels per batch = [one, r0..31, one, r32..63]
    w1e = np.zeros((H + 1, K2), np.float32)
    w1e[:, 1 : H + 1] = 0.5 * W1[:, :H]
    w1e[:, H + 2 :] = 0.5 * W1[:, H:]
    w2hv = (0.5 * W2).astype(np.float32)
    bt = np.zeros((11, FS), np.float32)
    for t in range(T):
        row = np.empty((B, K2), np.float32)
        row[:, 0] = 30.0
        row[:, H + 1] = 30.0
        row[:, 1 : H + 1] = (t + 1.0) * b1[:H]
        row[:, H + 2 :] = (t + 1.0) * b1[H:]
        bt[t] = row.reshape(-1)
    bt[10, : B * H] = np.tile(10.0 * b2, B)

    in_maps = []
    for c in range(NC):
        rows = _rows_of(c)
        msc = np.ascontiguousarray(
            mst[:, :, rows].reshape(T, NT, 128, RPC).transpose(1, 0, 2, 3)
        )
        in_maps.append(
            dict(
                mstack=msc,
                c1all=c1all,
                w1h=w1e,
                w2h=w2hv,
                biastab=bt,
            )
        )
    return in_maps


def kernel(inputs, states, dtw, spec_lap, laplacian, time_delay,
           W1, b1, W2, b2):
    states = np.asarray(states, np.float32)
    in_maps = make_in_maps(
        inputs, states, dtw, spec_lap, laplacian, time_delay, W1, b1, W2, b2
    )
    nc = _get_nc()
    res = run_bass_kernel_spmd(nc, in_maps, CORES, trace=False)

    cmat = np.empty((N, B, H), np.float32)
    umat = np.empty((2048, B, 2 * H), np.float32)
    keep = np.r_[1 : H + 1, H + 2 : K2]
    for c in range(NC):
        rows = _rows_of(c)
        cmat[rows] = np.asarray(
            res.results[c]["hout"], np.float32
        ).reshape(RPC, B, H)
        s9 = np.asarray(res.results[c]["sig9"], np.float32).reshape(256, B, K2)
        umat[256 * c : 256 * c + 256] = s9[:, :, keep]
    u = umat.transpose(1, 0, 2).reshape(B, NH)
    cfl = cmat.transpose(1, 0, 2).reshape(B, NH)
    h9 = states[T - 1]
    return (u * h9 + (1.0 - u) * cfl).astype(np.float32)


# revision 20
# speedup vs baseline: 1.1124x; 1.1124x over previous
"""FESTGCN Trainium2 kernel: 8-core SPMD Bass/Tile implementation (v4).

Algorithm (validated against the reference in numpy, see sim_check.py):
  For t in 0..9:
    M_t = dtw * (ceil|td| > 9-t) + (spec_lap + I)   [t=9: + laplacian, x2/3]
    S1 += M_t @ c1_t,   c1_t = [x_t | h_t]
    gcn1_t = 0.5 * S1 @ W1 + (t+1) b1 ;  sig_t = sigmoid(gcn1_t)
    r_t = first flat half of sig_t ;  c2_t = [x_t | r_t*h_t]
    S2 += M_t @ c2_t
  u = second flat half of sig_9 ; c = tanh(0.5 * S2 @ W2 + 10 b2)
  out = u*h_9 + (1-u)*c        (final mix done on host)

Structure:
  * All ten masked matrices M_t are baked on the HOST (fp16, transposed and
    sliced per core, laid out [nt, t, 128, 512] so two consecutive steps load
    as one 256KB DMA) - TRN2's vector engine is far too slow to mask per step.
  * Contraction axis in sigma order (even c2-nodes first); each core owns rows
    pi_c = [256c,+256) u [2048+256c,+256), so the per-step AllGather carries
    only the r-side sigmoids and every gather read is a contiguous row slice.
  * The sigmoid vector is widened 64->66 per batch with two always-1.0
    channels (bias 30 through sigmoid), so c2 = [x | r*h] is ONE elementwise
    multiply of the gathered gate slice with the c1 tile - no extra x column
    handling on any engine.
  * Single interleaved pass: conv2(t) runs DELAY=2 outer steps behind
    conv1(t); M/c1 tiles are held in SBUF across the window, AllGather latency
    hides under the intervening compute.  A dummy AllGather up front absorbs
    the collective entry barrier.
  * Final gating mix on host (no aux collective, no redundant full output).
"""

import numpy as np

import concourse.bacc as bacc
import concourse.mybir as mybir
import concourse.tile as tile
from concourse.bass_utils import run_bass_kernel_spmd

B, T, N, H = 4, 10, 4096, 32
NC = 8
RPC = N // NC            # 512 rows per core
NT = N // 128            # 32 contraction tiles
MT = RPC // 128          # 4 m-tiles per core
F1 = B * (H + 1)         # 132 moving columns per conv
K2 = 2 * H + 2           # 66: widened sigmoid channels per batch
FS = B * K2              # 264
NH = N * H
DELAY = 2                # conv2 runs this many outer steps behind conv1
f32 = mybir.dt.float32
f16 = mybir.dt.float16
Alu = mybir.AluOpType
Act = mybir.ActivationFunctionType
CORES = list(range(NC))

PERM = np.concatenate([np.arange(0, N, 2), np.arange(1, N, 2)])
IPERM = np.empty(N, np.int64)
IPERM[PERM] = np.arange(N)


def _rows_of(c):
    return np.concatenate(
        [np.arange(256 * c, 256 * c + 256),
         np.arange(2048 + 256 * c, 2048 + 256 * c + 256)]
    )


def _build_nc():
    nc = bacc.Bacc(
        "TRN2",
        target_bir_lowering=False,
        debug=False,
        enable_asserts=False,
        num_devices=NC,
    )
    mstack = nc.dram_tensor("mstack", [NT, T, 128, RPC], f16,
                            kind="ExternalInput").ap()
    c1all = nc.dram_tensor("c1all", [T, N, F1], f16, kind="ExternalInput").ap()
    w1h = nc.dram_tensor("w1h", [H + 1, K2], f32, kind="ExternalInput").ap()
    w2h = nc.dram_tensor("w2h", [H + 1, H], f32, kind="ExternalInput").ap()
    biastab = nc.dram_tensor("biastab", [11, FS], f32, kind="ExternalInput").ap()
    hout = nc.dram_tensor("hout", [RPC, B * H], f32, kind="ExternalOutput").ap()
    sig9 = nc.dram_tensor("sig9", [256, FS], f16, kind="ExternalOutput").ap()

    with tile.TileContext(nc) as tc:
        with (
            tc.tile_pool(name="msp", bufs=48) as msp,       # M step-pair tiles
            tc.tile_pool(name="c1p", bufs=100) as c1p,      # c1 tiles, 3-step hold
            tc.tile_pool(name="rlp", bufs=34) as rlp,       # gathered sigmoids
            tc.tile_pool(name="c2p", bufs=34) as c2p,
            tc.tile_pool(name="sm", bufs=1) as sm,
            tc.tile_pool(name="acc", bufs=2) as accp,
            tc.tile_pool(name="wk", bufs=3) as wk,
            tc.tile_pool(name="zp", bufs=1, space="PSUM") as zp,
            tc.tile_pool(name="tpz", bufs=2, space="PSUM") as tpzp,
            tc.tile_pool(name="g1p", bufs=2, space="PSUM") as g1p,
            tc.tile_pool(name="dramp", bufs=1, space="DRAM") as dramp,
        ):
            agsrc = [
                dramp.tile([256, FS], f16, tag=f"agsrc{t}", name=f"agsrc{t}")
                for t in range(T)
            ]
            agdst = [
                dramp.tile([NC * 256, FS], f16, tag=f"agdst{t}",
                           name=f"agdst{t}", addr_space="Shared")
                for t in range(T)
            ]
            agwsrc = dramp.tile([8, 8], f16, tag="agwsrc", name="agwsrc")
            agwdst = dramp.tile([NC * 8, 8], f16, tag="agwdst", name="agwdst",
                                addr_space="Shared")

            # ---------------- prologue ----------------
            iota_i = wk.tile([128, 128], mybir.dt.int32, tag="iota", bufs=1)
            nc.gpsimd.iota(iota_i[:], pattern=[[1, 128]], base=0,
                           channel_multiplier=-1)
            ident = sm.tile([128, 128], f32, tag="ident")
            nc.vector.tensor_scalar(ident[:], iota_i[:], 0, None,
                                    op0=Alu.is_equal)
            w1s = sm.tile([H + 1, K2], f32, tag="w1s")
            nc.sync.dma_start(w1s[:], w1h[:])
            w2s = sm.tile([H + 1, H], f32, tag="w2s")
            nc.sync.dma_start(w2s[:], w2h[:])
            wup = wk.tile([8, 8], f16, tag="wup", bufs=1)
            nc.vector.memset(wup[:], 0.0)
            nc.sync.dma_start(agwsrc[:], wup[:])
            # dummy collective: absorbs the CC entry barrier before step 0
            nc.gpsimd.collective_compute(
                "AllGather", Alu.bypass, replica_groups=[CORES],
                ins=[agwsrc[:]], outs=[agwdst[:]],
            )

            s1 = [accp.tile([128, F1], f32, tag=f"s1_{mt}", name=f"s1_{mt}")
                  for mt in range(MT)]
            s2 = [accp.tile([128, F1], f32, tag=f"s2_{mt}", name=f"s2_{mt}")
                  for mt in range(MT)]
            for mt in range(MT):
                nc.vector.memset(s1[mt][:], 0.0)
                nc.vector.memset(s2[mt][:], 0.0)

            ms_hold = {}
            c1_hold = {}
            rl_hold = {}
            c2_hold = {}

            def chain1(t, s1t):
                """S1(t) -> gcn1 -> sigmoid (widened) -> agsrc[t]/AG."""
                mts = (0, 1, 2, 3) if t == T - 1 else (0, 1)
                biasf = wk.tile([128, FS], f32, tag="biasf", bufs=2)
                nc.sync.dma_start(
                    biasf[:], biastab[t : t + 1, :].broadcast_to((128, FS))
                )
                for mt in mts:
                    tb = wk.tile([128, F1], f32, tag="tbf", bufs=3)
                    nc.vector.tensor_copy(tb[:], s1t[mt][:])
                    g1 = g1p.tile([128, FS], f32, tag="g1")
                    for b in range(B):
                        tz = tpzp.tile([H + 1, 128], f32, tag="tz")
                        nc.tensor.transpose(
                            tz[:], tb[:, b * (H + 1) : (b + 1) * (H + 1)],
                            ident[:],
                        )
                        zbt = wk.tile([H + 1, 128], f32, tag="zbt", bufs=2)
                        nc.scalar.copy(zbt[:], tz[:])
                        nc.tensor.matmul(
                            g1[:, b * K2 : (b + 1) * K2], zbt[:], w1s[:],
                            start=True, stop=True,
                        )
                    sigi = wk.tile([128, FS], f32, tag="sigi", bufs=2)
                    nc.vector.scalar_tensor_tensor(
                        sigi[:], g1[:], 1.0, biasf[:], op0=Alu.mult, op1=Alu.add
                    )
                    sigb = wk.tile([128, FS], f16, tag="sigb", bufs=3)
                    nc.scalar.activation(sigb[:], sigi[:], Act.Sigmoid)
                    if mt < 2:
                        nc.sync.dma_start(
                            agsrc[t][mt * 128 : (mt + 1) * 128, :], sigb[:]
                        )
                    else:
                        nc.sync.dma_start(
                            sig9[(mt - 2) * 128 : (mt - 1) * 128, :], sigb[:]
                        )
                nc.gpsimd.collective_compute(
                    "AllGather",
                    Alu.bypass,
                    replica_groups=[CORES],
                    ins=[agsrc[t][:]],
                    outs=[agdst[t][:]],
                )

            def fetch_rl(t):
                # One outer step after AG(t) fired, so the scalar queue's
                # semaphore wait on the collective is already satisfied.
                rls = []
                for gt in range(16):
                    r = rlp.tile([128, FS], f16, tag="rl", name="rl")
                    nc.scalar.dma_start(
                        r[:], agdst[t][gt * 128 : (gt + 1) * 128, :]
                    )
                    rls.append(r)
                rl_hold[t] = rls

            # ---------------- interleaved main loop ----------------
            for s in range(T + DELAY):
                if s >= DELAY:
                    # c2 gate multiplies on DVE, ahead of everything so the
                    # conv2 matmuls can chase conv1's without a PE stall
                    t2 = s - DELAY
                    rls = rl_hold.pop(t2)
                    c1s = c1_hold[t2]
                    c2s = []
                    for nt in range(NT):
                        gt, p = (nt, 0) if nt < 16 else (nt - 16, 1)
                        c2 = c2p.tile([128, F1], f16, tag="c2", name="c2")
                        nc.vector.tensor_mul(
                            c2.rearrange("p (b k) -> p b k", k=H + 1),
                            rls[gt].rearrange("p (b k) -> p b k", k=K2)[
                                :, :, 33 * p : 33 * p + 33
                            ],
                            c1s[nt].rearrange("p (b k) -> p b k", k=H + 1),
                        )
                        c2s.append(c2)
                    c2_hold[t2] = c2s
                if s < T:
                    t = s
                    za = zp.tile([128, 2 * F1], f32, tag="z1a", name=f"z1a_{t}")
                    zb = zp.tile([128, 2 * F1], f32, tag="z1b", name=f"z1b_{t}")
                    zh = [za, zb]
                    if t % 2 == 0:
                        mss = []
                        for nt in range(NT):
                            m = msp.tile([128, 2 * RPC], f16, tag="ms",
                                         name="ms")
                            eng = nc.sync
                            eng.dma_start(
                                m.rearrange("p (two m) -> p two m", two=2),
                                mstack[nt, t : t + 2, :, :].rearrange(
                                    "two p m -> p two m"
                                ),
                            )
                            mss.append(m)
                        ms_hold[t] = mss
                        ms_hold[t + 1] = mss
                    mss = ms_hold[t]
                    moff = (t % 2) * RPC
                    c1s = []
                    for nt in range(NT):
                        c1 = c1p.tile([128, F1], f16, tag="c1", name="c1")
                        nc.scalar.dma_start(
                            c1[:], c1all[t, nt * 128 : (nt + 1) * 128, :]
                        )
                        c1s.append(c1)
                        for mt in range(MT):
                            nc.tensor.matmul(
                                zh[mt // 2][:, (mt % 2) * F1 : (mt % 2 + 1) * F1],
                                mss[nt][:, moff + mt * 128 : moff + (mt + 1) * 128],
                                c1[:],
                                start=(nt == 0 and mt % 2 == 0),
                                stop=(nt == NT - 1),
                            )
                    c1_hold[t] = c1s
                    s1t = []
                    for mt in range(MT):
                        s1n = accp.tile([128, F1], f32, tag=f"s1_{mt}")
                        nc.vector.tensor_add(
                            s1n[:], s1[mt][:],
                            zh[mt // 2][:, (mt % 2) * F1 : (mt % 2 + 1) * F1],
                        )
                        s1[mt] = s1n
                        s1t.append(s1n)

                if s >= DELAY:
                    # conv2 matmuls immediately chase conv1's on the PE
                    t2 = s - DELAY
                    za = zp.tile([128, 2 * F1], f32, tag="z2a", name=f"z2a_{t2}")
                    zb = zp.tile([128, 2 * F1], f32, tag="z2b", name=f"z2b_{t2}")
                    zh = [za, zb]
                    mss = ms_hold.pop(t2)
                    moff = (t2 % 2) * RPC
                    c1_hold.pop(t2)
                    c2s = c2_hold.pop(t2)
                    for nt in range(NT):
                        for mt in range(MT):
                            nc.tensor.matmul(
                                zh[mt // 2][:, (mt % 2) * F1 : (mt % 2 + 1) * F1],
                                mss[nt][:, moff + mt * 128 : moff + (mt + 1) * 128],
                                c2s[nt][:],
                                start=(nt == 0 and mt % 2 == 0),
                                stop=(nt == NT - 1),
                            )
                    for mt in range(MT):
                        s2n = accp.tile([128, F1], f32, tag=f"s2_{mt}")
                        nc.vector.tensor_add(
                            s2n[:], s2[mt][:],
                            zh[mt // 2][:, (mt % 2) * F1 : (mt % 2 + 1) * F1],
                        )
                        s2[mt] = s2n

                if s < T:
                    chain1(s, s1t)
                if 1 <= s <= T:
                    fetch_rl(s - 1)

            # ---------------- tail ----------------
            bias2f = wk.tile([128, B * H], f32, tag="bias2f", bufs=1)
            nc.sync.dma_start(
                bias2f[:], biastab[10 : 11, : B * H].broadcast_to((128, B * H))
            )
            for mt in range(MT):
                tb2 = wk.tile([128, F1], f32, tag="tbf", bufs=3)
                nc.vector.tensor_copy(tb2[:], s2[mt][:])
                g2 = g1p.tile([128, FS], f32, tag="g1")
                for b in range(B):
                    tz = tpzp.tile([H + 1, 128], f32, tag="tz")
                    nc.tensor.transpose(
                        tz[:], tb2[:, b * (H + 1) : (b + 1) * (H + 1)], ident[:]
                    )
                    zbt = wk.tile([H + 1, 128], f32, tag="zbt", bufs=2)
                    nc.scalar.copy(zbt[:], tz[:])
                    nc.tensor.matmul(
                        g2[:, b * H : (b + 1) * H], zbt[:], w2s[:],
                        start=True, stop=True,
                    )
                tani = wk.tile([128, B * H], f32, tag="tani", bufs=2)
                nc.vector.scalar_tensor_tensor(
                    tani[:], g2[:, : B * H], 1.0, bias2f[:],
                    op0=Alu.mult, op1=Alu.add,
                )
                tanf = wk.tile([128, B * H], f32, tag="tanf", bufs=2)
                nc.scalar.activation(tanf[:], tani[:], Act.Tanh)
                nc.sync.dma_start(hout[mt * 128 : (mt + 1) * 128, :], tanf[:])

    nc.finalize()
    return nc


_NC_CACHE = None


def _get_nc():
    global _NC_CACHE
    if _NC_CACHE is None:
        _NC_CACHE = _build_nc()
    return _NC_CACHE


def make_in_maps(inputs, states, dtw, spec_lap, laplacian, time_delay,
                 W1, b1, W2, b2):
    inputs = np.asarray(inputs, np.float32)
    states = np.asarray(states, np.float32)
    dtw = np.asarray(dtw, np.float32)
    spec_lap = np.asarray(spec_lap, np.float32)
    laplacian = np.asarray(laplacian, np.float32)
    time_delay = np.asarray(time_delay, np.float32)
    W1 = np.asarray(W1, np.float32)
    b1 = np.asarray(b1, np.float32)
    W2 = np.asarray(W2, np.float32)
    b2 = np.asarray(b2, np.float32)

    ct_full = np.ceil(np.abs(time_delay))
    Gdtw = np.ascontiguousarray(dtw[:, PERM].T)
    Gct = np.ascontiguousarray(ct_full[:, PERM].T)
    Gsle = spec_lap[:, PERM].T.copy()
    Gsle[IPERM, np.arange(N)] += 1.0
    Glap = laplacian[:, PERM].T

    scratch = np.empty((N, N), np.float32)
    mst = np.empty((T, N, N), np.float16)
    for t in range(T - 1):
        np.multiply(Gdtw, (Gct > np.float32(9 - t)), out=scratch)
        scratch += Gsle
        mst[t] = scratch
    np.multiply(Gdtw, (Gct >= np.float32(1.0)), out=scratch)
    scratch += Gsle
    scratch += Glap
    mst[T - 1] = scratch

    x = inputs.transpose(1, 0, 2)               # [T, B, N]
    h = states.reshape(T, B, N, H)
    conc = np.concatenate([x[:, :, :, None], h], axis=3)  # [T,B,N,33]
    conc[9] *= 2.0 / 3.0
    concp = conc.transpose(0, 2, 1, 3)[:, PERM]  # [T, N(sigma), B, 33]
    c1all = np.ascontiguousarray(concp.reshape(T, N, F1), np.float16)

    # widened W1 / biases: channels per batch = [one, r0..31, one, r32..63]
    w1e = np.zeros((H + 1, K2), np.float32)
    w1e[:, 1 : H + 1] = 0.5 * W1[:, :H]
    w1e[:, H + 2 :] = 0.5 * W1[:, H:]
    w2hv = (0.5 * W2).astype(np.float32)
    bt = np.zeros((11, FS), np.float32)
    for t in range(T):
        row = np.empty((B, K2), np.float32)
        row[:, 0] = 30.0
        row[:, H + 1] = 30.0
        row[:, 1 : H + 1] = (t + 1.0) * b1[:H]
        row[:, H + 2 :] = (t + 1.0) * b1[H:]
        bt[t] = row.reshape(-1)
    bt[10, : B * H] = np.tile(10.0 * b2, B)

    in_maps = []
    for c in range(NC):
        rows = _rows_of(c)
        msc = np.ascontiguousarray(
            mst[:, :, rows].reshape(T, NT, 128, RPC).transpose(1, 0, 2, 3)
        )
        in_maps.append(
            dict(
                mstack=msc,
                c1all=c1all,
                w1h=w1e,
                w2h=w2hv,
                biastab=bt,
            )
        )
    return in_maps


def kernel(inputs, states, dtw, spec_lap, laplacian, time_delay,
           W1, b1, W2, b2):
    states = np.asarray(states, np.float32)
    in_maps = make_in_maps(
        inputs, states, dtw, spec_lap, laplacian, time_delay, W1, b1, W2, b2
    )
    nc = _get_nc()
    res = run_bass_kernel_spmd(nc, in_maps, CORES, trace=False)

    cmat = np.empty((N, B, H), np.float32)
    umat = np.empty((2048, B, 2 * H), np.float32)
    keep = np.r_[1 : H + 1, H + 2 : K2]
    for c in range(NC):
        rows = _rows_of(c)
        cmat[rows] = np.asarray(
            res.results[c]["hout"], np.float32
        ).reshape(RPC, B, H)
        s9 = np.asarray(res.results[c]["sig9"], np.float32).reshape(256, B, K2)
        umat[256 * c : 256 * c + 256] = s9[:, :, keep]
    u = umat.transpose(1, 0, 2).reshape(B, NH)
    cfl = cmat.transpose(1, 0, 2).reshape(B, NH)
    h9 = states[T - 1]
    return (u * h9 + (1.0 - u) * cfl).astype(np.float32)
